# revision 19
# baseline (speedup 1.0000x reference)
"""Multi-head attention kernel for 8 Trainium2 NeuronCores.

Problem: B=4, S=2048, D=1024, H=16, Dh=64 MHA with key-side boolean mask.

Sharding: core c handles (batch b = c//2, head-half g = c%2, 8 heads each).
QKV are column-parallel, the output projection is row-parallel (Megatron
style); the host sums the two partial output projections per batch and adds
the output bias.

Host-side preprocessing (pure data marshalling):
  - All inputs are pre-tiled into DMA-native layouts (partition-major,
    contiguous per partition) and cast to bf16 (input rounding only; all
    accumulation stays fp32 in PSUM — end-to-end relmax ~2e-3 vs 2e-2 gate).
  - x is transposed per batch (the PE contracts over the partition dim).
  - Keys with mask=False contribute exactly zero after softmax, so the host
    gathers only the unmasked keys (padded to a multiple of 384 with zero
    rows whose exp-bias is -1e30 => exp == 0 exactly).

On-core dataflow:
  xT --(Wk,Wv)--> KT[f,k] bf16 [128 = head-pair features], Vau[k,f] (+biases)
  xT --(Wq)--> QT[f,q] bf16
  per query-chunk qc (512 queries), per head-pair t:
    scores: row-tiled matmul pair (head A on PE rows 0:63, head B rows
      64:127) into one PSUM tile [128,1024] -> ONE ScalarE exp for both.
    out_aug[65,q] = [V_h | ones]^T x E   (row 64 = softmax denominator)
    normalize: DVE reciprocal of the den row, GpSimd partition_broadcast,
      DVE multiply (PSUM operand) -> attnT[f,q] f32r.
  after the 4 head-pairs of qc: output projection for those 512 rows,
  ScalarE drain (ScalarE is idle at the chunk boundary; keeping these
  drains off DVE lets the attn PSUM tiles recycle without queueing
  behind the normalize chain).

PSUM budget (8 banks): scores 2x[128,1024] double-buffered (4) + a shared
4x[*,512] pool for the out_aug accumulators and output-projection tiles.
ScalarE's exp stream (144 x [128,1024] per core) is the critical resource;
everything else is arranged to stay off it or under it.
"""

import os
import numpy as np
import ml_dtypes

os.environ.setdefault("MYCRO_LOCAL_CACHE", "1")

BF16 = ml_dtypes.bfloat16
D_MODEL = 1024
N_HEADS = 16
D_HEAD = 64
BATCH = 4
SEQ = 2048
N_CORES = 8
FH = 512          # features per core (8 heads x 64)
HPC = 8           # heads per core
NEG = -1.0e30     # additive bias for padded/masked keys; exp -> 0 exactly

_COMPILED = {}    # k_pad -> nc
last_results = None  # BassKernelResults of the most recent run (for test.py)


def _build(k_pad):
    """Emit + compile the per-core bass kernel for a given padded key count."""
    import concourse.bacc as bacc
    import concourse.bass as bass
    import concourse.tile as tile
    from concourse import mybir

    f32 = mybir.dt.float32
    f32r = mybir.dt.float32r
    bf16 = mybir.dt.bfloat16
    KT_N = k_pad // 128                     # number of 128-key tiles
    KC = 512 if k_pad % 512 == 0 else 384   # key-side chunk
    assert k_pad % KC == 0 and KC % 128 == 0
    NKC = k_pad // KC

    nc = bacc.Bacc("TRN2", target_bir_lowering=False, debug=False,
                   num_devices=N_CORES)

    # all pre-tiled on host into DMA-native layouts (bf16)
    dxq = nc.dram_tensor("xq", [4, 128, 8, 512], bf16, kind="ExternalInput")
    dxk = nc.dram_tensor("xk", [NKC, 128, 8, KC], bf16, kind="ExternalInput")
    dWq = nc.dram_tensor("Wq", [128, 8, FH], bf16, kind="ExternalInput")
    dWk = nc.dram_tensor("Wk", [128, 8, FH], bf16, kind="ExternalInput")
    dWv = nc.dram_tensor("Wv", [128, 8, FH], bf16, kind="ExternalInput")
    dWo = nc.dram_tensor("Wo", [128, 4, D_MODEL], f32r, kind="ExternalInput")
    dbc = nc.dram_tensor("bcst", [128, 8 + KT_N], f32, kind="ExternalInput")
    dbv = nc.dram_tensor("bv", [FH], bf16, kind="ExternalInput")
    dcst = nc.dram_tensor("consts", [256], bf16, kind="ExternalInput")  # ones
    dout = nc.dram_tensor("out", [SEQ, D_MODEL], f32, kind="ExternalOutput")

    EXP = mybir.ActivationFunctionType.Exp
    IDn = mybir.ActivationFunctionType.Identity

    with tile.TileContext(nc) as tc:
        with tc.tile_pool(name="persist", bufs=1) as pers:
            # ---- constants in SBUF ----
            bc = pers.tile([128, 8 + KT_N], f32, tag="bcst")
            nc.sync.dma_start(out=bc, in_=dbc.ap())
            bq = bc[:, 0:4]
            bk = bc[:, 4:8]
            mb = bc[:, 8:8 + KT_N]
            bv_row = pers.tile([1, FH], bf16, tag="bvr")
            nc.sync.dma_start(out=bv_row, in_=dbv.ap()[None, :])
            ones_t = pers.tile([1, 128], bf16, tag="ones")
            nc.sync.dma_start(out=ones_t, in_=dcst.ap()[None, 0:128])
            ones128 = ones_t[:, :]

            # ---- persistent activations ----
            QT = pers.tile([128, 4, SEQ], bf16, tag="QT")        # [f, q]
            KT = pers.tile([128, 4, k_pad], bf16, tag="KT")      # [f, k]
            Vau = pers.tile([128, KT_N, HPC, 65], f32r, tag="Vau")
            # the 65th column of every head is the constant softmax-
            # denominator weight: set once, never projected.
            nc.vector.memset(Vau[:, :, :, 64].bitcast(f32), 1.0)

            # ================= projections =================
            wtq_cm = tc.tile_pool(name="wtq", bufs=1)
            wtq = wtq_cm.__enter__()
            xqp_cm = tc.tile_pool(name="xq", bufs=2)
            xqp = xqp_cm.__enter__()
            ppool_cm = tc.tile_pool(name="pp", bufs=3, space="PSUM")
            ppool = ppool_cm.__enter__()

            # ----- K side (KT, V) -----
            with tc.tile_pool(name="wtk", bufs=1) as wtk, \
                 tc.tile_pool(name="xk", bufs=2) as xkp:
                pk = ppool
                wk = wtk.tile([128, 8, FH], bf16, tag="wk")
                nc.sync.dma_start(out=wk, in_=dWk.ap())
                wv = wtk.tile([128, 8, FH], bf16, tag="wv")
                nc.sync.dma_start(out=wv, in_=dWv.ap())
                for kc in range(NKC):
                    xk_t = xkp.tile([128, 8, KC], bf16, tag="xk")
                    nc.sync.dma_start(out=xk_t, in_=dxk.ap()[kc])
                    for ft in range(4):
                        ps = pk.tile([128, KC], f32, tag="pk")
                        for dt in range(8):
                            nc.tensor.matmul(
                                ps,
                                lhsT=wk[:, dt, ft * 128:(ft + 1) * 128],
                                rhs=xk_t[:, dt, :],
                                start=(dt == 0), stop=(dt == 7))
                        ks = slice(kc * KC, (kc + 1) * KC)
                        nc.scalar.activation(KT[:, ft, ks], ps, IDn,
                                             bias=bk[:, ft:ft + 1])
                    for kb in range(KC // 128):
                        kg = kc * (KC // 128) + kb
                        ps = pk.tile([128, FH], f32, tag="pk")
                        for dt in range(8):
                            nc.tensor.matmul(
                                ps,
                                lhsT=xk_t[:, dt, kb * 128:(kb + 1) * 128],
                                rhs=wv[:, dt, :],
                                start=(dt == 0), stop=False)
                        nc.tensor.matmul(ps, lhsT=ones128,
                                         rhs=bv_row,
                                         start=False, stop=True)
                        nc.scalar.copy(Vau[:, kg, :, 0:64], ps)

            # ----- Q side (QT) -----
            wq = wtq.tile([128, 8, FH], bf16, tag="wq")
            nc.sync.dma_start(out=wq, in_=dWq.ap())
            for qc in range(4):
                xq_t = xqp.tile([128, 8, 512], bf16, tag="xq")
                nc.sync.dma_start(out=xq_t, in_=dxq.ap()[qc])
                for ft in range(4):
                    ps = ppool.tile([128, 512], f32, tag="pk")
                    for dt in range(8):
                        nc.tensor.matmul(
                            ps,
                            lhsT=wq[:, dt, ft * 128:(ft + 1) * 128],
                            rhs=xq_t[:, dt, :],
                            start=(dt == 0), stop=(dt == 7))
                    nc.scalar.activation(QT[:, ft, qc * 512:(qc + 1) * 512],
                                         ps, IDn, bias=bq[:, ft:ft + 1])

            ppool_cm.__exit__(None, None, None)

            # ====== attention core + interleaved Q/output projection ======
            att2_cm = tc.tile_pool(name="att2", bufs=1)
            att2 = att2_cm.__enter__()
            attnT = att2.tile([128, 4, SEQ], f32r, tag="attnT")  # [f, q]
            wo = att2.tile([128, 4, D_MODEL], f32r, tag="wo")
            nc.sync.dma_start(out=wo, in_=dWo.ap())
            with tc.tile_pool(name="et", bufs=6) as etp, \
                 tc.tile_pool(name="rp", bufs=6) as rpp, \
                 tc.tile_pool(name="bp", bufs=6) as bcp, \
                 tc.tile_pool(name="ot", bufs=4) as otp, \
                 tc.tile_pool(name="sp", bufs=2, space="PSUM") as sp, \
                 tc.tile_pool(name="av", bufs=4, space="PSUM") as avp:

                def attn_block(qc, t):
                    """scores -> exp -> attn*V for one head pair / q chunk."""
                    qs = slice(qc * 512, (qc + 1) * 512)
                    avA = avp.tile([65, 512], f32, tag="av")
                    avB = avp.tile([65, 512], f32, tag="av")
                    for kt in range(KT_N):
                        kts = slice(kt * 128, (kt + 1) * 128)
                        sAB = sp.tile([128, 1024], f32, tag="s")
                        # row-tiled pair: head A in PE rows 0:63, head B in
                        # rows 64:127, one PSUM tile -> one fused exp.
                        nc.tensor.matmul(
                            sAB[:, 0:512],
                            lhsT=KT[0:64, t, kts], rhs=QT[0:64, t, qs],
                            start=True, stop=True)
                        nc.tensor.matmul(
                            sAB[:, 512:1024],
                            lhsT=KT[64:128, t, kts], rhs=QT[64:128, t, qs],
                            start=True, stop=True)
                        eAB = etp.tile([128, 1024], f32r, tag="et")
                        nc.scalar.activation(eAB, sAB, EXP,
                                             bias=mb[:, kt:kt + 1],
                                             scale=0.125)
                        nc.tensor.matmul(
                            avA, lhsT=Vau[:, kt, 2 * t, :],
                            rhs=eAB[:, 0:512],
                            start=(kt == 0), stop=(kt == KT_N - 1))
                        nc.tensor.matmul(
                            avB, lhsT=Vau[:, kt, 2 * t + 1, :],
                            rhs=eAB[:, 512:1024],
                            start=(kt == 0), stop=(kt == KT_N - 1))
                    # normalize: attnT = out_aug[0:64] * bcast(1/den)
                    for h, av in ((0, avA), (1, avB)):
                        r = rpp.tile([1, 512], f32, tag="r")
                        nc.vector.reciprocal(r, av[64:65, :])
                        bcast = bcp.tile([64, 512], f32, tag="bc")
                        nc.gpsimd.partition_broadcast(bcast, r)
                        nc.vector.tensor_mul(attnT[64 * h:64 * (h + 1), t, qs],
                                             av[0:64, :], bcast)

                def outproj_super(qc, st4):
                    """output projection of 128 of qc's rows (both D halves
                    in one 2-bank tile from the score pool — free at chunk
                    boundaries, so this never touches the av pool)."""
                    st = qc * 4 + st4
                    ss = slice(st * 128, (st + 1) * 128)
                    ps = sp.tile([128, 1024], f32, tag="s")
                    for dh in range(2):
                        for ft in range(4):
                            nc.tensor.matmul(
                                ps[:, dh * 512:(dh + 1) * 512],
                                lhsT=attnT[:, ft, ss],
                                rhs=wo[:, ft, dh * 512:(dh + 1) * 512],
                                start=(ft == 0), stop=(ft == 3))
                    ot = otp.tile([128, 1024], f32, tag="ot")
                    nc.scalar.copy(ot, ps)
                    nc.sync.dma_start(out=dout.ap()[ss, :], in_=ot)

                for qc in range(4):
                    for t in range(4):
                        attn_block(qc, t)
                        # previous chunk's output projection lands after the
                        # next chunk's first head-pair: by then its normalize
                        # is long done, and the exp stream never waits on it.
                        if qc > 0 and t == 0:
                            for st4 in range(4):
                                outproj_super(qc - 1, st4)
                for st4 in range(4):
                    outproj_super(3, st4)
            att2_cm.__exit__(None, None, None)
            xqp_cm.__exit__(None, None, None)
            wtq_cm.__exit__(None, None, None)

    nc.compile()
    return nc


def _get_compiled(k_pad):
    if k_pad not in _COMPILED:
        _COMPILED[k_pad] = _build(k_pad)
    return _COMPILED[k_pad]


def _tile_pf(a, p=128):
    """[P*t, f...] -> contiguous [p, t, f...] partition-major tiling."""
    t = a.shape[0] // p
    return np.ascontiguousarray(
        a.reshape(t, p, *a.shape[1:]).swapaxes(0, 1))


def _prep_core_inputs(x, attention_mask, Wq, bq, Wk, bk, Wv, bv, Wo):
    """Host-side shard prep. Returns (in_maps, k_pad)."""
    x = np.asarray(x, np.float32)
    mask = np.asarray(attention_mask, bool)
    idxs = [np.nonzero(mask[b])[0] for b in range(BATCH)]
    ke_max = max(1, max(len(i) for i in idxs))
    k_pad = 384 * ((ke_max + 383) // 384)
    if k_pad > SEQ:
        k_pad = SEQ
    KC = 512 if k_pad % 512 == 0 else 384
    NKC = k_pad // KC
    KT_N = k_pad // 128

    consts = np.zeros(256, np.float32)
    consts[0:128] = 1.0

    in_maps = []
    for b in range(BATCH):
        xT = x[b].T                                  # [D, S] view
        # xq: [qc, p, dt, 512]
        xq = np.ascontiguousarray(
            xT.reshape(8, 128, 4, 512).transpose(2, 1, 0, 3)).astype(BF16)
        idx = idxs[b]
        ke = len(idx)
        if ke > k_pad:
            idx = idx[:k_pad]
            ke = k_pad
        xkT = np.zeros((D_MODEL, k_pad), np.float32)
        xkT[:, :ke] = x[b][idx].T
        # xk: [kc, p, dt, KC]
        xk = np.ascontiguousarray(
            xkT.reshape(8, 128, NKC, KC).transpose(2, 1, 0, 3)).astype(BF16)
        maskb = np.zeros(k_pad, np.float32)
        maskb[ke:] = NEG
        mb_t = _tile_pf(maskb)                       # [128, KT_N]
        for g in range(2):
            fs = slice(g * FH, (g + 1) * FH)
            in_maps.append({
                "xq": xq,
                "xk": xk,
                "Wq": _tile_pf(np.asarray(Wq[:, fs], np.float32)).astype(BF16),
                "Wk": _tile_pf(np.asarray(Wk[:, fs], np.float32)).astype(BF16),
                "Wv": _tile_pf(np.asarray(Wv[:, fs], np.float32)).astype(BF16),
                "Wo": _tile_pf(np.asarray(Wo[fs, :], np.float32)),
                "bcst": np.concatenate(
                    [_tile_pf(np.asarray(bq[fs], np.float32)),
                     _tile_pf(np.asarray(bk[fs], np.float32)),
                     mb_t], axis=1).astype(np.float32),
                "bv": np.asarray(bv[fs], np.float32).astype(BF16),
                "consts": consts.astype(BF16),
            })
    return in_maps, k_pad


def kernel(x, attention_mask, Wq, bq, Wk, bk, Wv, bv, Wo, bo):
    global last_results
    from concourse.bass_utils import run_bass_kernel_spmd

    in_maps, k_pad = _prep_core_inputs(x, attention_mask, Wq, bq, Wk, bk, Wv, bv, Wo)
    nc = _get_compiled(k_pad)
    res = run_bass_kernel_spmd(nc, in_maps, core_ids=list(range(N_CORES)))
    last_results = res

    bo = np.asarray(bo, np.float32)
    out = np.empty((BATCH, SEQ, D_MODEL), np.float32)
    for b in range(BATCH):
        out[b] = res.results[2 * b]["out"] + res.results[2 * b + 1]["out"] + bo
    return out
